# revision 11
# baseline (speedup 1.0000x reference)
"""Trainium2 Bass kernel for nn_AbstractAffine (DeepPoly-style backsubstitution).

Math
----
Reference scans L=16 layers over relational bound state (xl, xu, bl, bu):
    pl = max(xl,0); nl = min(xl,0); pu = max(xu,0); nu = min(xu,0)
    bl += pl@c_lo + nl@c_up ;  bu += pu@c_up + nu@c_lo
    xl  = pl@A_lo + nl@A_up ;  xu  = pu@A_up + nu@A_lo
Using max(x,0)=(x+|x|)/2, min(x,0)=(x-|x|)/2 with S=(A_lo+A_up)/2,
Dm=(A_lo-A_up)/2:
    xl' = xl@S + |xl|@Dm ;  xu' = xu@S - |xu|@Dm
(same form for the bias vectors and for the final input-bound reduction).

Mapping
-------
Output-neuron rows are sharded over 8 cores (128 rows each). Each core keeps
its state TRANSPOSED (contraction index j on partitions), packed per j-chunk
as (128, 512) slices [xlT | xuT | |xl|T | -|xu|T] of one per-layer state tile.
Every matmul uses an A-matrix slice (natural HBM layout) as the stationary
operand and the packed state as the moving operand (free dim 256), so no
on-chip transposes are needed. A-matrices and state are fp16 (adds ~8e-4 rel
err), accumulation in fp32 PSUM.

Layer 0 exploits xl == xu == W^T: it computes P1 = x0@S and P2 = |x0|@Dm with
128-wide moving operands (half the PE work of a generic layer) in two phases
of 8 PSUM banks each, then forms xl1 = P1+P2, xu1 = P1-P2 on DVE. The initial
state is built on-chip from a 0.25 MB DMA of W columns (abs computed on the
scalar engine), instead of DMAing the 1 MB packed state.

The final bound reduction runs as a single PSUM accumulation group (bias row
pre-seeded into the DVE bias accumulator, fp16 ones-matmul partition-reduce,
16 input-bound matmuls interleaved with the last layer's staggered closes).
"""

import numpy as np
from contextlib import ExitStack

import concourse.bass as bass
import concourse.tile as tile
from concourse import bacc, mybir
from concourse.bass_utils import run_bass_kernel_spmd

L = 16
D = 1024
NCORES = 8
R = D // NCORES          # rows per core
JC = D // 128            # j-chunks
SDW = 2 * L + 3          # scdc columns per j-chunk (sc/dc pairs, s_in, d_in, -dc0)
WARMUP = 12

A_DT = mybir.dt.float16
A_NP = np.float16

_CACHE = {}


def _build():
    f32 = mybir.dt.float32
    nc = bacc.Bacc(None, target_bir_lowering=False)
    s_dram = nc.dram_tensor("s_mats", [L, 128, JC * 1024], A_DT, kind="ExternalInput")
    d_dram = nc.dram_tensor("d_mats", [L, 128, JC * 1024], A_DT, kind="ExternalInput")
    scdc_dram = nc.dram_tensor("scdc", [128, JC * SDW], A_DT, kind="ExternalInput")
    t0_dram = nc.dram_tensor("t0cols", [128, 1024], A_DT, kind="ExternalInput")
    bias_dram = nc.dram_tensor("bias0", [1, 256], f32, kind="ExternalInput")
    out_dram = nc.dram_tensor("out", [1, 256], f32, kind="ExternalOutput")

    MULT = mybir.AluOpType.mult
    MIN = mybir.AluOpType.min
    MAX = mybir.AluOpType.max
    ADD = mybir.AluOpType.add
    ABS = mybir.ActivationFunctionType.Abs

    with tile.TileContext(nc) as tc:
        with ExitStack() as ctx:
            apool = ctx.enter_context(tc.tile_pool(name="amat", bufs=4))
            spool = ctx.enter_context(tc.tile_pool(name="state", bufs=2))
            cpool = ctx.enter_context(tc.tile_pool(name="consts", bufs=1))
            ppool = ctx.enter_context(tc.tile_pool(name="psum", bufs=1, space="PSUM"))

            # PE warm-up: dummy matmuls on a memset tile into a trash PSUM
            # bank while the first DMAs are in flight — releases the HAM
            # clock throttle before the first real matmul arrives. memset on
            # gpsimd: that queue is live earliest, so the PE starts sooner.
            dummy = cpool.tile([128, 256], A_DT, tag="dummy")
            nc.gpsimd.memset(dummy[:], 0.0)
            trash = ppool.tile([128, 256], f32, tag="ps7")
            for i in range(WARMUP):
                nc.tensor.matmul(trash[:], dummy[:, 0:128], dummy[:],
                                 start=(i == 0), stop=(i == WARMUP - 1))

            # DMA triggers cost ~600ns of sequencer time each (DIRECT2D
            # descriptor generation), so the two HWDGE rings split the work:
            # sync carries the s-matrices, scalar carries W-columns + the
            # d-matrices. Everything else stays off those two queues early.
            t0 = cpool.tile([128, 2048], A_DT, tag="t0")
            nc.scalar.dma_start(t0[:, 0:1024], t0_dram[:])
            s0 = apool.tile([128, JC * 1024], A_DT, tag="s", name="s0")
            for h in range(8):
                nc.sync.dma_start(s0[:, h * 1024:(h + 1) * 1024],
                                  s_dram[0, :, h * 1024:(h + 1) * 1024])
            d0 = apool.tile([128, JC * 1024], A_DT, tag="d", name="d0")
            for h in range(8):
                nc.scalar.dma_start(d0[:, h * 1024:(h + 1) * 1024],
                                    d_dram[0, :, h * 1024:(h + 1) * 1024])
            scdc_all = cpool.tile([128, JC * SDW], A_DT, tag="scdc")
            nc.scalar.dma_start(scdc_all[:], scdc_dram[:])
            scdc = [scdc_all[:, jc * SDW:(jc + 1) * SDW] for jc in range(JC)]

            # bias accumulator: ACC[p, :] += sc[p]*state[p, :] + dc[p]*abs[p, :]
            # summed over all layers on DVE (axpys run during the matmul
            # phase). The output bias row is seeded into partition 0, so the
            # final fp16 ones-matmul partition-reduce yields bounds + bias in
            # one shot. Keeps all PE cycles for coefficient matmuls.
            acc = cpool.tile([128, 256], f32, tag="acc")
            nc.vector.memset(acc[:], 0.0)
            nc.scalar.dma_start(acc[0:1, :], bias_dram[:])
            ones16 = cpool.tile([128, 1], A_DT, tag="ones")
            nc.vector.memset(ones16[:], 1.0)
            acc16 = cpool.tile([128, 256], A_DT, tag="acc16")
            outsb = cpool.tile([1, 256], f32, tag="outsb")

            # |x0|T = max(x0, -x0) built on the DVE (idle early, and off the
            # trigger-carrying queues)
            for jc in range(JC):
                nc.vector.scalar_tensor_tensor(
                    t0[:, 1024 + jc * 128:1024 + (jc + 1) * 128],
                    t0[:, jc * 128:(jc + 1) * 128], -1.0,
                    t0[:, jc * 128:(jc + 1) * 128], MULT, MAX)

            # layer 0, phase A: P1[cc] = x0 @ S (128-wide moving operands)
            T1 = spool.tile([128, JC * 512], A_DT, tag="T", name="T1")
            psA = [ppool.tile([128, 128], f32, tag=f"ps{cc}", name=f"psA{cc}")
                   for cc in range(8)]
            for jc in range(JC):
                for cc in range(8):
                    nc.tensor.matmul(
                        psA[cc][:], s0[:, jc * 1024 + cc * 128:jc * 1024 + cc * 128 + 128],
                        t0[:, jc * 128:(jc + 1) * 128],
                        start=(jc == 0), stop=(jc == JC - 1))
            for cc in range(8):
                nc.scalar.copy(T1[:, cc * 512:cc * 512 + 128], psA[cc][:])
            # phase B: P2[cc] = |x0| @ Dm, then xl1 = P1+P2, xu1 = P1-P2
            psB = [ppool.tile([128, 128], f32, tag=f"ps{cc}", name=f"psB{cc}")
                   for cc in range(8)]
            for jc in range(JC):
                for cc in range(8):
                    nc.tensor.matmul(
                        psB[cc][:], d0[:, jc * 1024 + cc * 128:jc * 1024 + cc * 128 + 128],
                        t0[:, 1024 + jc * 128:1024 + (jc + 1) * 128],
                        start=(jc == 0), stop=(jc == JC - 1))
            for cc in range(8):
                nt = T1[:, cc * 512:(cc + 1) * 512]
                nc.vector.tensor_sub(nt[:, 128:256], nt[:, 0:128], psB[cc][:])
                nc.vector.tensor_add(nt[:, 0:128], nt[:, 0:128], psB[cc][:])
                nc.scalar.activation(nt[:, 256:384], nt[:, 0:128], ABS)
                nc.vector.scalar_tensor_tensor(
                    nt[:, 384:512], nt[:, 128:256], -1.0, nt[:, 128:256], MULT, MIN)
            T = T1

            # layer-0 bias round: xl = xu = x0, so 128-wide axpys with the
            # shared x0/|x0| inputs (col 2L+2 holds -dc0 for the upper half).
            # Emitted after the phase-B combines so the DVE queue doesn't
            # stall on the scdc arrival ahead of them.
            for jc in range(JC):
                x0s = t0[:, jc * 128:(jc + 1) * 128]
                abss = t0[:, 1024 + jc * 128:1024 + (jc + 1) * 128]
                nc.vector.scalar_tensor_tensor(
                    acc[:, 0:128], x0s, scdc[jc][:, 0:1], acc[:, 0:128], MULT, ADD)
                nc.vector.scalar_tensor_tensor(
                    acc[:, 128:256], x0s, scdc[jc][:, 0:1], acc[:, 128:256], MULT, ADD)
                nc.vector.scalar_tensor_tensor(
                    acc[:, 0:128], abss, scdc[jc][:, 1:2], acc[:, 0:128], MULT, ADD)
                nc.vector.scalar_tensor_tensor(
                    acc[:, 128:256], abss, scdc[jc][:, 2 * L + 2:2 * L + 3],
                    acc[:, 128:256], MULT, ADD)

            for l in range(1, L):
                s_t = apool.tile([128, JC * 1024], A_DT, tag="s", name=f"s{l}")
                d_t = apool.tile([128, JC * 1024], A_DT, tag="d", name=f"d{l}")
                # quarter-granularity (4 KB per partition row) measures the
                # best per-engine DMA bandwidth; s on the sync ring, d on
                # the scalar ring so descriptor generation runs in parallel
                for q in range(4):
                    nc.sync.dma_start(s_t[:, q * 2048:(q + 1) * 2048],
                                      s_dram[l, :, q * 2048:(q + 1) * 2048])
                    nc.scalar.dma_start(d_t[:, q * 2048:(q + 1) * 2048],
                                        d_dram[l, :, q * 2048:(q + 1) * 2048])

                for jc in range(JC):
                    Ts = T[:, jc * 512:(jc + 1) * 512]
                    nc.vector.scalar_tensor_tensor(
                        acc[:], Ts[:, 0:256], scdc[jc][:, 2 * l:2 * l + 1],
                        acc[:], MULT, ADD)
                    nc.vector.scalar_tensor_tensor(
                        acc[:], Ts[:, 256:512], scdc[jc][:, 2 * l + 1:2 * l + 2],
                        acc[:], MULT, ADD)

                last = (l == L - 1)
                if last:
                    # acc is complete once this layer's bias round retires;
                    # the fp16 copy feeds the cheap ones-matmul reduce
                    nc.vector.tensor_copy(acc16[:], acc[:])

                newT = spool.tile([128, JC * 512], A_DT, tag="T", name=f"T{l + 1}")
                ps = [ppool.tile([128, 256], f32, tag=f"ps{i}", name=f"ps{i}_{l}")
                      for i in range(8)]
                # jc-outer for all but the last j-chunk: consumes DMA chunks
                # as they land, all 8 cc accumulation groups open in their
                # own PSUM banks
                for jc in range(JC - 1):
                    for cc in range(8):
                        off = jc * 1024 + cc * 128
                        nc.tensor.matmul(
                            ps[cc][:], s_t[:, off:off + 128],
                            T[:, jc * 512:jc * 512 + 256], start=(jc == 0), stop=False)
                        nc.tensor.matmul(
                            ps[cc][:], d_t[:, off:off + 128],
                            T[:, jc * 512 + 256:(jc + 1) * 512], start=False, stop=False)
                if last:
                    pb = ppool.tile([1, 256], f32, tag="ps0", name="pb")
                # last j-chunk cc-outer: staggers group closes so PSUM->SBUF
                # copies overlap the remaining matmuls; on the final layer
                # the input-bound matmuls interleave into the close stream
                jc = JC - 1
                for cc in range(8):
                    off = jc * 1024 + cc * 128
                    nc.tensor.matmul(
                        ps[cc][:], s_t[:, off:off + 128],
                        T[:, jc * 512:jc * 512 + 256], start=False, stop=False)
                    nc.tensor.matmul(
                        ps[cc][:], d_t[:, off:off + 128],
                        T[:, jc * 512 + 256:(jc + 1) * 512], start=False, stop=True)
                    nt = newT[:, cc * 512:(cc + 1) * 512]
                    if last:
                        nc.vector.tensor_copy(nt[:, 0:256], ps[cc][:])
                    else:
                        nc.scalar.copy(nt[:, 0:256], ps[cc][:])
                    nc.scalar.activation(nt[:, 256:384], nt[:, 0:128], ABS)
                    nc.vector.scalar_tensor_tensor(
                        nt[:, 384:512], nt[:, 128:256], -1.0, nt[:, 128:256],
                        MULT, MIN)
                    if last:
                        if cc == 0:
                            nc.tensor.matmul(pb[:], ones16[:], acc16[:],
                                             start=True, stop=False)
                        else:
                            c = cc - 1
                            nc.tensor.matmul(
                                pb[:], scdc[c][:, 2 * L:2 * L + 1],
                                newT[:, c * 512:c * 512 + 256],
                                start=False, stop=False)
                            nc.tensor.matmul(
                                pb[:], scdc[c][:, 2 * L + 1:2 * L + 2],
                                newT[:, c * 512 + 256:(c + 1) * 512],
                                start=False, stop=False)
                T = newT

            c = JC - 1
            nc.tensor.matmul(pb[:], scdc[c][:, 2 * L:2 * L + 1],
                             T[:, c * 512:c * 512 + 256], start=False, stop=False)
            nc.tensor.matmul(pb[:], scdc[c][:, 2 * L + 1:2 * L + 2],
                             T[:, c * 512 + 256:(c + 1) * 512], start=False, stop=True)
            nc.vector.tensor_copy(outsb[:], pb[:])
            nc.sync.dma_start(out_dram[:], outsb[:])
    nc.compile()
    return nc


def _prep_inputs(weights, biases, net_x_lowers, net_x_uppers,
                 net_b_lowers, net_b_uppers, input_lowers, input_uppers):
    W = np.ascontiguousarray(np.asarray(weights, dtype=np.float32))
    b = np.asarray(biases, dtype=np.float32).reshape(D)
    AL = np.asarray(net_x_lowers, dtype=np.float32)
    AU = np.asarray(net_x_uppers, dtype=np.float32)
    cL = np.asarray(net_b_lowers, dtype=np.float32).reshape(L, D)
    cU = np.asarray(net_b_uppers, dtype=np.float32).reshape(L, D)
    lo = np.asarray(input_lowers, dtype=np.float32).reshape(D)
    up = np.asarray(input_uppers, dtype=np.float32).reshape(D)

    S = 0.5 * (AL + AU)
    Dm = 0.5 * (AL - AU)
    # (L, 128, JC*1024): [l, p, jc*1024 + c] = S[l, jc*128 + p, c]
    s_mats = np.ascontiguousarray(
        S.reshape(L, JC, 128, D).transpose(0, 2, 1, 3).reshape(L, 128, JC * D)
    ).astype(A_NP)
    d_mats = np.ascontiguousarray(
        Dm.reshape(L, JC, 128, D).transpose(0, 2, 1, 3).reshape(L, 128, JC * D)
    ).astype(A_NP)

    sc = 0.5 * (cL + cU)
    dc = 0.5 * (cL - cU)
    s_in = 0.5 * (lo + up)
    d_in = 0.5 * (lo - up)
    sd = np.empty((JC, 128, SDW), np.float32)
    sd[:, :, 0:2 * L:2] = sc.reshape(L, JC, 128).transpose(1, 2, 0)
    sd[:, :, 1:2 * L:2] = dc.reshape(L, JC, 128).transpose(1, 2, 0)
    sd[:, :, 2 * L] = s_in.reshape(JC, 128)
    sd[:, :, 2 * L + 1] = d_in.reshape(JC, 128)
    sd[:, :, 2 * L + 2] = -dc[0].reshape(JC, 128)
    # dram layout (128, JC*SDW): [p, jc*SDW + col]
    scdc = np.ascontiguousarray(
        sd.transpose(1, 0, 2).reshape(128, JC * SDW)).astype(A_NP)

    Wh = W.astype(A_NP)  # x0T[j, r] = W[j, r]; round once
    Wr = Wh.reshape(JC, 128, D)
    in_maps = []
    for k in range(NCORES):
        # (128, 1024): [p, jc*128 + r] = W[jc*128 + p, k*R + r]
        cols = np.ascontiguousarray(
            Wr[:, :, k * R:(k + 1) * R].transpose(1, 0, 2).reshape(128, JC * R))
        b0 = np.empty((1, 256), np.float32)
        b0[0, 0:128] = b[k * R:(k + 1) * R]
        b0[0, 128:256] = b[k * R:(k + 1) * R]
        in_maps.append({
            "s_mats": s_mats,
            "d_mats": d_mats,
            "scdc": scdc,
            "t0cols": cols,
            "bias0": b0,
        })
    return in_maps


def _run(inputs, trace=False):
    if "nc" not in _CACHE:
        _CACHE["nc"] = _build()
    nc = _CACHE["nc"]
    in_maps = _prep_inputs(**inputs)
    try:
        res = run_bass_kernel_spmd(nc, in_maps, core_ids=list(range(NCORES)),
                                   trace=trace)
    except Exception:
        # transient NRT device errors have been observed; retry once
        res = run_bass_kernel_spmd(nc, in_maps, core_ids=list(range(NCORES)),
                                   trace=trace)
    lowers = np.empty((D, 1), np.float32)
    uppers = np.empty((D, 1), np.float32)
    for k in range(NCORES):
        arr = res.results[k]["out"]
        lowers[k * R:(k + 1) * R, 0] = arr[0, 0:128]
        uppers[k * R:(k + 1) * R, 0] = arr[0, 128:256]
    out = np.stack([lowers, uppers])
    return out, res


def kernel(**inputs):
    out, _ = _run(inputs, trace=False)
    return out


# revision 13
# speedup vs baseline: 1.0515x; 1.0515x over previous
"""Trainium2 Bass kernel for nn_AbstractAffine (DeepPoly-style backsubstitution).

Math
----
Reference scans L=16 layers over relational bound state (xl, xu, bl, bu):
    pl = max(xl,0); nl = min(xl,0); pu = max(xu,0); nu = min(xu,0)
    bl += pl@c_lo + nl@c_up ;  bu += pu@c_up + nu@c_lo
    xl  = pl@A_lo + nl@A_up ;  xu  = pu@A_up + nu@A_lo
Using max(x,0)=(x+|x|)/2, min(x,0)=(x-|x|)/2 with S=(A_lo+A_up)/2,
Dm=(A_lo-A_up)/2:
    xl' = xl@S + |xl|@Dm ;  xu' = xu@S - |xu|@Dm
(same form for the bias vectors and for the final input-bound reduction).

Mapping
-------
Output-neuron rows are sharded over 8 cores (128 rows each). Each core keeps
its state TRANSPOSED (contraction index j on partitions), packed per j-chunk
as (128, 512) slices [xlT | xuT | |xl|T | -|xu|T] of one per-layer state tile.
Every matmul uses an A-matrix slice (natural HBM layout) as the stationary
operand and the packed state as the moving operand (free dim 256), so no
on-chip transposes are needed. A-matrices and state are fp16 (adds ~8e-4 rel
err), accumulation in fp32 PSUM.

Layer 0 exploits xl == xu == W^T: it computes P1 = x0@S and P2 = |x0|@Dm with
128-wide moving operands (half the PE work of a generic layer) in two phases
of 8 PSUM banks each, then forms xl1 = P1+P2, xu1 = P1-P2 on DVE. The initial
state is built on-chip from a 0.25 MB DMA of W columns (abs computed on the
scalar engine), instead of DMAing the 1 MB packed state.

The final bound reduction runs as a single PSUM accumulation group (bias row
pre-seeded into the DVE bias accumulator, fp16 ones-matmul partition-reduce,
16 input-bound matmuls interleaved with the last layer's staggered closes).
"""

import numpy as np
from contextlib import ExitStack

import concourse.bass as bass
import concourse.tile as tile
from concourse import bacc, mybir
from concourse.bass_utils import run_bass_kernel_spmd

L = 16
D = 1024
NCORES = 8
R = D // NCORES          # rows per core
JC = D // 128            # j-chunks
SDW = 2 * L + 3          # scdc columns per j-chunk (sc/dc pairs, s_in, d_in, -dc0)
WARMUP = 12

A_DT = mybir.dt.float16
A_NP = np.float16

_CACHE = {}


def _build():
    f32 = mybir.dt.float32
    nc = bacc.Bacc(None, target_bir_lowering=False)
    s_dram = nc.dram_tensor("s_mats", [L, 128, JC * 1024], A_DT, kind="ExternalInput")
    d_dram = nc.dram_tensor("d_mats", [L, 128, JC * 1024], A_DT, kind="ExternalInput")
    scdc_dram = nc.dram_tensor("scdc", [128, JC * SDW], A_DT, kind="ExternalInput")
    t0_dram = nc.dram_tensor("t0cols", [128, 1024], A_DT, kind="ExternalInput")
    bias_dram = nc.dram_tensor("bias0", [1, 256], f32, kind="ExternalInput")
    out_dram = nc.dram_tensor("out", [1, 256], f32, kind="ExternalOutput")

    MULT = mybir.AluOpType.mult
    MIN = mybir.AluOpType.min
    MAX = mybir.AluOpType.max
    ADD = mybir.AluOpType.add
    ABS = mybir.ActivationFunctionType.Abs

    with tile.TileContext(nc) as tc:
        with ExitStack() as ctx:
            apool = ctx.enter_context(tc.tile_pool(name="amat", bufs=4))
            spool = ctx.enter_context(tc.tile_pool(name="state", bufs=2))
            cpool = ctx.enter_context(tc.tile_pool(name="consts", bufs=1))
            ppool = ctx.enter_context(tc.tile_pool(name="psum", bufs=1, space="PSUM"))

            # PE warm-up: dummy matmuls on a memset tile into a trash PSUM
            # bank while the first DMAs are in flight — releases the HAM
            # clock throttle before the first real matmul arrives. memset on
            # gpsimd: that queue is live earliest, so the PE starts sooner.
            dummy = cpool.tile([128, 256], A_DT, tag="dummy")
            nc.gpsimd.memset(dummy[:], 0.0)
            trash = ppool.tile([128, 256], f32, tag="ps7")
            for i in range(WARMUP):
                nc.tensor.matmul(trash[:], dummy[:, 0:128], dummy[:],
                                 start=(i == 0), stop=(i == WARMUP - 1))

            # DMA triggers cost ~600ns of sequencer time each (DIRECT2D
            # descriptor generation), so the two HWDGE rings split the work:
            # sync carries the s-matrices, scalar carries W-columns + the
            # d-matrices. Everything else stays off those two queues early.
            t0 = cpool.tile([128, 2048], A_DT, tag="t0")
            nc.scalar.dma_start(t0[:, 0:1024], t0_dram[:])
            s0 = apool.tile([128, JC * 1024], A_DT, tag="s", name="s0")
            for h in range(4):
                nc.sync.dma_start(s0[:, h * 2048:(h + 1) * 2048],
                                  s_dram[0, :, h * 2048:(h + 1) * 2048])
            d0 = apool.tile([128, JC * 1024], A_DT, tag="d", name="d0")
            for h in range(4):
                nc.sync.dma_start(d0[:, h * 2048:(h + 1) * 2048],
                                  d_dram[0, :, h * 2048:(h + 1) * 2048])
            scdc_all = cpool.tile([128, JC * SDW], A_DT, tag="scdc")
            nc.scalar.dma_start(scdc_all[:], scdc_dram[:])
            scdc = [scdc_all[:, jc * SDW:(jc + 1) * SDW] for jc in range(JC)]

            # bias accumulator: ACC[p, :] += sc[p]*state[p, :] + dc[p]*abs[p, :]
            # summed over all layers on DVE (axpys run during the matmul
            # phase). The output bias row is seeded into partition 0, so the
            # final fp16 ones-matmul partition-reduce yields bounds + bias in
            # one shot. Keeps all PE cycles for coefficient matmuls.
            acc = cpool.tile([128, 256], f32, tag="acc")
            nc.vector.memset(acc[:], 0.0)
            nc.scalar.dma_start(acc[0:1, :], bias_dram[:])
            ones16 = cpool.tile([128, 1], A_DT, tag="ones")
            nc.vector.memset(ones16[:], 1.0)
            acc16 = cpool.tile([128, 256], A_DT, tag="acc16")
            outsb = cpool.tile([1, 256], f32, tag="outsb")

            # |x0|T = max(x0, -x0) built on the DVE (idle early, and off the
            # trigger-carrying queues)
            for jc in range(JC):
                nc.vector.scalar_tensor_tensor(
                    t0[:, 1024 + jc * 128:1024 + (jc + 1) * 128],
                    t0[:, jc * 128:(jc + 1) * 128], -1.0,
                    t0[:, jc * 128:(jc + 1) * 128], MULT, MAX)

            # layer 0, phase A: P1[cc] = x0 @ S (128-wide moving operands)
            T1 = spool.tile([128, JC * 512], A_DT, tag="T", name="T1")
            psA = [ppool.tile([128, 128], f32, tag=f"ps{cc}", name=f"psA{cc}")
                   for cc in range(8)]
            for jc in range(JC):
                for cc in range(8):
                    nc.tensor.matmul(
                        psA[cc][:], s0[:, jc * 1024 + cc * 128:jc * 1024 + cc * 128 + 128],
                        t0[:, jc * 128:(jc + 1) * 128],
                        start=(jc == 0), stop=(jc == JC - 1))
            for cc in range(8):
                nc.scalar.copy(T1[:, cc * 512:cc * 512 + 128], psA[cc][:])
            # phase B: P2[cc] = |x0| @ Dm, then xl1 = P1+P2, xu1 = P1-P2
            psB = [ppool.tile([128, 128], f32, tag=f"ps{cc}", name=f"psB{cc}")
                   for cc in range(8)]
            for jc in range(JC):
                for cc in range(8):
                    nc.tensor.matmul(
                        psB[cc][:], d0[:, jc * 1024 + cc * 128:jc * 1024 + cc * 128 + 128],
                        t0[:, 1024 + jc * 128:1024 + (jc + 1) * 128],
                        start=(jc == 0), stop=(jc == JC - 1))
            for cc in range(8):
                nt = T1[:, cc * 512:(cc + 1) * 512]
                nc.vector.tensor_sub(nt[:, 128:256], nt[:, 0:128], psB[cc][:])
                nc.vector.tensor_add(nt[:, 0:128], nt[:, 0:128], psB[cc][:])
                nc.scalar.activation(nt[:, 256:384], nt[:, 0:128], ABS)
                nc.vector.scalar_tensor_tensor(
                    nt[:, 384:512], nt[:, 128:256], -1.0, nt[:, 128:256], MULT, MIN)
            T = T1

            # layer-0 bias round: xl = xu = x0, so 128-wide axpys with the
            # shared x0/|x0| inputs (col 2L+2 holds -dc0 for the upper half).
            # Emitted after the phase-B combines so the DVE queue doesn't
            # stall on the scdc arrival ahead of them.
            for jc in range(JC):
                x0s = t0[:, jc * 128:(jc + 1) * 128]
                abss = t0[:, 1024 + jc * 128:1024 + (jc + 1) * 128]
                nc.vector.scalar_tensor_tensor(
                    acc[:, 0:128], x0s, scdc[jc][:, 0:1], acc[:, 0:128], MULT, ADD)
                nc.vector.scalar_tensor_tensor(
                    acc[:, 128:256], x0s, scdc[jc][:, 0:1], acc[:, 128:256], MULT, ADD)
                nc.vector.scalar_tensor_tensor(
                    acc[:, 0:128], abss, scdc[jc][:, 1:2], acc[:, 0:128], MULT, ADD)
                nc.vector.scalar_tensor_tensor(
                    acc[:, 128:256], abss, scdc[jc][:, 2 * L + 2:2 * L + 3],
                    acc[:, 128:256], MULT, ADD)

            for l in range(1, L):
                s_t = apool.tile([128, JC * 1024], A_DT, tag="s", name=f"s{l}")
                d_t = apool.tile([128, JC * 1024], A_DT, tag="d", name=f"d{l}")
                # quarter-granularity (4 KB per partition row) measures the
                # best per-engine DMA bandwidth; both streams stay on the
                # sync ring — it carries nothing else, so triggers keep a
                # multi-layer prefetch lead
                for q in range(4):
                    nc.sync.dma_start(s_t[:, q * 2048:(q + 1) * 2048],
                                      s_dram[l, :, q * 2048:(q + 1) * 2048])
                    nc.sync.dma_start(d_t[:, q * 2048:(q + 1) * 2048],
                                      d_dram[l, :, q * 2048:(q + 1) * 2048])

                for jc in range(JC):
                    Ts = T[:, jc * 512:(jc + 1) * 512]
                    nc.vector.scalar_tensor_tensor(
                        acc[:], Ts[:, 0:256], scdc[jc][:, 2 * l:2 * l + 1],
                        acc[:], MULT, ADD)
                    nc.vector.scalar_tensor_tensor(
                        acc[:], Ts[:, 256:512], scdc[jc][:, 2 * l + 1:2 * l + 2],
                        acc[:], MULT, ADD)

                last = (l == L - 1)
                if last:
                    # acc is complete once this layer's bias round retires;
                    # the fp16 copy feeds the cheap ones-matmul reduce
                    nc.vector.tensor_copy(acc16[:], acc[:])

                newT = spool.tile([128, JC * 512], A_DT, tag="T", name=f"T{l + 1}")
                ps = [ppool.tile([128, 256], f32, tag=f"ps{i}", name=f"ps{i}_{l}")
                      for i in range(8)]
                # jc-outer for all but the last j-chunk: consumes DMA chunks
                # as they land, all 8 cc accumulation groups open in their
                # own PSUM banks
                for jc in range(JC - 1):
                    for cc in range(8):
                        off = jc * 1024 + cc * 128
                        nc.tensor.matmul(
                            ps[cc][:], s_t[:, off:off + 128],
                            T[:, jc * 512:jc * 512 + 256], start=(jc == 0), stop=False)
                        nc.tensor.matmul(
                            ps[cc][:], d_t[:, off:off + 128],
                            T[:, jc * 512 + 256:(jc + 1) * 512], start=False, stop=False)
                if last:
                    pb = ppool.tile([1, 256], f32, tag="ps0", name="pb")
                # last j-chunk cc-outer: staggers group closes so PSUM->SBUF
                # copies overlap the remaining matmuls; on the final layer
                # the input-bound matmuls interleave into the close stream
                jc = JC - 1
                for cc in range(8):
                    off = jc * 1024 + cc * 128
                    nc.tensor.matmul(
                        ps[cc][:], s_t[:, off:off + 128],
                        T[:, jc * 512:jc * 512 + 256], start=False, stop=False)
                    nc.tensor.matmul(
                        ps[cc][:], d_t[:, off:off + 128],
                        T[:, jc * 512 + 256:(jc + 1) * 512], start=False, stop=True)
                    nt = newT[:, cc * 512:(cc + 1) * 512]
                    if last:
                        nc.vector.tensor_copy(nt[:, 0:256], ps[cc][:])
                    else:
                        nc.scalar.copy(nt[:, 0:256], ps[cc][:])
                    nc.scalar.activation(nt[:, 256:384], nt[:, 0:128], ABS)
                    nc.vector.scalar_tensor_tensor(
                        nt[:, 384:512], nt[:, 128:256], -1.0, nt[:, 128:256],
                        MULT, MIN)
                    if last:
                        if cc == 0:
                            nc.tensor.matmul(pb[:], ones16[:], acc16[:],
                                             start=True, stop=False)
                        else:
                            c = cc - 1
                            nc.tensor.matmul(
                                pb[:], scdc[c][:, 2 * L:2 * L + 1],
                                newT[:, c * 512:c * 512 + 256],
                                start=False, stop=False)
                            nc.tensor.matmul(
                                pb[:], scdc[c][:, 2 * L + 1:2 * L + 2],
                                newT[:, c * 512 + 256:(c + 1) * 512],
                                start=False, stop=False)
                T = newT

            c = JC - 1
            nc.tensor.matmul(pb[:], scdc[c][:, 2 * L:2 * L + 1],
                             T[:, c * 512:c * 512 + 256], start=False, stop=False)
            nc.tensor.matmul(pb[:], scdc[c][:, 2 * L + 1:2 * L + 2],
                             T[:, c * 512 + 256:(c + 1) * 512], start=False, stop=True)
            nc.vector.tensor_copy(outsb[:], pb[:])
            nc.sync.dma_start(out_dram[:], outsb[:])
    nc.compile()
    return nc


def _prep_inputs(weights, biases, net_x_lowers, net_x_uppers,
                 net_b_lowers, net_b_uppers, input_lowers, input_uppers):
    W = np.ascontiguousarray(np.asarray(weights, dtype=np.float32))
    b = np.asarray(biases, dtype=np.float32).reshape(D)
    AL = np.asarray(net_x_lowers, dtype=np.float32)
    AU = np.asarray(net_x_uppers, dtype=np.float32)
    cL = np.asarray(net_b_lowers, dtype=np.float32).reshape(L, D)
    cU = np.asarray(net_b_uppers, dtype=np.float32).reshape(L, D)
    lo = np.asarray(input_lowers, dtype=np.float32).reshape(D)
    up = np.asarray(input_uppers, dtype=np.float32).reshape(D)

    S = 0.5 * (AL + AU)
    Dm = 0.5 * (AL - AU)
    # (L, 128, JC*1024): [l, p, jc*1024 + c] = S[l, jc*128 + p, c]
    s_mats = np.ascontiguousarray(
        S.reshape(L, JC, 128, D).transpose(0, 2, 1, 3).reshape(L, 128, JC * D)
    ).astype(A_NP)
    d_mats = np.ascontiguousarray(
        Dm.reshape(L, JC, 128, D).transpose(0, 2, 1, 3).reshape(L, 128, JC * D)
    ).astype(A_NP)

    sc = 0.5 * (cL + cU)
    dc = 0.5 * (cL - cU)
    s_in = 0.5 * (lo + up)
    d_in = 0.5 * (lo - up)
    sd = np.empty((JC, 128, SDW), np.float32)
    sd[:, :, 0:2 * L:2] = sc.reshape(L, JC, 128).transpose(1, 2, 0)
    sd[:, :, 1:2 * L:2] = dc.reshape(L, JC, 128).transpose(1, 2, 0)
    sd[:, :, 2 * L] = s_in.reshape(JC, 128)
    sd[:, :, 2 * L + 1] = d_in.reshape(JC, 128)
    sd[:, :, 2 * L + 2] = -dc[0].reshape(JC, 128)
    # dram layout (128, JC*SDW): [p, jc*SDW + col]
    scdc = np.ascontiguousarray(
        sd.transpose(1, 0, 2).reshape(128, JC * SDW)).astype(A_NP)

    Wh = W.astype(A_NP)  # x0T[j, r] = W[j, r]; round once
    Wr = Wh.reshape(JC, 128, D)
    in_maps = []
    for k in range(NCORES):
        # (128, 1024): [p, jc*128 + r] = W[jc*128 + p, k*R + r]
        cols = np.ascontiguousarray(
            Wr[:, :, k * R:(k + 1) * R].transpose(1, 0, 2).reshape(128, JC * R))
        b0 = np.empty((1, 256), np.float32)
        b0[0, 0:128] = b[k * R:(k + 1) * R]
        b0[0, 128:256] = b[k * R:(k + 1) * R]
        in_maps.append({
            "s_mats": s_mats,
            "d_mats": d_mats,
            "scdc": scdc,
            "t0cols": cols,
            "bias0": b0,
        })
    return in_maps


def _run(inputs, trace=False):
    if "nc" not in _CACHE:
        _CACHE["nc"] = _build()
    nc = _CACHE["nc"]
    in_maps = _prep_inputs(**inputs)
    try:
        res = run_bass_kernel_spmd(nc, in_maps, core_ids=list(range(NCORES)),
                                   trace=trace)
    except Exception:
        # transient NRT device errors have been observed; retry once
        res = run_bass_kernel_spmd(nc, in_maps, core_ids=list(range(NCORES)),
                                   trace=trace)
    lowers = np.empty((D, 1), np.float32)
    uppers = np.empty((D, 1), np.float32)
    for k in range(NCORES):
        arr = res.results[k]["out"]
        lowers[k * R:(k + 1) * R, 0] = arr[0, 0:128]
        uppers[k * R:(k + 1) * R, 0] = arr[0, 128:256]
    out = np.stack([lowers, uppers])
    return out, res


def kernel(**inputs):
    out, _ = _run(inputs, trace=False)
    return out


# revision 24
# speedup vs baseline: 1.0714x; 1.0189x over previous
"""Trainium2 Bass kernel for nn_AbstractAffine (DeepPoly-style backsubstitution).

Math
----
Reference scans L=16 layers over relational bound state (xl, xu, bl, bu):
    pl = max(xl,0); nl = min(xl,0); pu = max(xu,0); nu = min(xu,0)
    bl += pl@c_lo + nl@c_up ;  bu += pu@c_up + nu@c_lo
    xl  = pl@A_lo + nl@A_up ;  xu  = pu@A_up + nu@A_lo
Using max(x,0)=(x+|x|)/2, min(x,0)=(x-|x|)/2 with S=(A_lo+A_up)/2,
Dm=(A_lo-A_up)/2:
    xl' = xl@S + |xl|@Dm ;  xu' = xu@S - |xu|@Dm
(same form for the bias vectors and for the final input-bound reduction).

Mapping
-------
Output-neuron rows are sharded over 8 cores (128 rows each). Each core keeps
its state TRANSPOSED (contraction index j on partitions), packed per j-chunk
as (128, 512) slices [xlT | xuT | |xl|T | -|xu|T] of one per-layer state tile.
Every matmul uses an A-matrix slice (natural HBM layout) as the stationary
operand and the packed state as the moving operand (free dim 256), so no
on-chip transposes are needed. A-matrices and state are fp16 (adds ~8e-4 rel
err), accumulation in fp32 PSUM.

Layer 0 exploits xl == xu == W^T: it computes P1 = x0@S and P2 = |x0|@Dm with
128-wide moving operands (half the PE work of a generic layer) in two phases
of 8 PSUM banks each, then forms xl1 = P1+P2, xu1 = P1-P2 on DVE. The initial
state is built on-chip from a 0.25 MB DMA of W columns (abs computed on the
scalar engine), instead of DMAing the 1 MB packed state.

The final bound reduction runs as a single PSUM accumulation group (bias row
pre-seeded into the DVE bias accumulator, fp16 ones-matmul partition-reduce,
16 input-bound matmuls interleaved with the last layer's staggered closes).
"""

import numpy as np
from contextlib import ExitStack

import concourse.bass as bass
import concourse.tile as tile
from concourse import bacc, mybir
from concourse.bass_utils import run_bass_kernel_spmd

L = 16
D = 1024
NCORES = 8
R = D // NCORES          # rows per core
JC = D // 128            # j-chunks
SDW = 2 * L + 3          # scdc columns per j-chunk (sc/dc pairs, s_in, d_in, -dc0)
WARMUP = 20
KFP8 = 2                 # layers 0..KFP8-1 ship their A-matrices as fp8 e3m4
F8_SCALE = 64.0          # power-of-two scale applied before the e3m4 cast

A_DT = mybir.dt.float16
A_NP = np.float16

_CACHE = {}


def _build():
    f32 = mybir.dt.float32
    f8 = mybir.dt.float8e3
    nc = bacc.Bacc(None, target_bir_lowering=False)
    # layers 0..KFP8-1 in fp8 e3m4 (scaled by F8_SCALE); the rest fp16.
    # The PE takes fp8 stationary with fp16 moving at full rate, so this
    # halves the DMA bytes exactly where the kernel is DMA-bound (the first
    # few layers, before the prefetch pipeline gets ahead of the PE).
    s8_dram = nc.dram_tensor("s8_mats", [KFP8, 128, JC * 1024], f8,
                             kind="ExternalInput")
    d8_dram = nc.dram_tensor("d8_mats", [KFP8, 128, JC * 1024], f8,
                             kind="ExternalInput")
    s_dram = nc.dram_tensor("s_mats", [L - KFP8, 128, JC * 1024], A_DT,
                            kind="ExternalInput")
    d_dram = nc.dram_tensor("d_mats", [L - KFP8, 128, JC * 1024], A_DT,
                            kind="ExternalInput")
    scdc_dram = nc.dram_tensor("scdc", [128, JC * SDW], A_DT, kind="ExternalInput")
    t0_dram = nc.dram_tensor("t0cols", [128, 1024], A_DT, kind="ExternalInput")
    bias_dram = nc.dram_tensor("bias0", [1, 256], f32, kind="ExternalInput")
    out_dram = nc.dram_tensor("out", [1, 256], f32, kind="ExternalOutput")

    MULT = mybir.AluOpType.mult
    MIN = mybir.AluOpType.min
    MAX = mybir.AluOpType.max
    ADD = mybir.AluOpType.add
    ABS = mybir.ActivationFunctionType.Abs

    with tile.TileContext(nc) as tc:
        with ExitStack() as ctx:
            apool = ctx.enter_context(tc.tile_pool(name="amat", bufs=4))
            spool = ctx.enter_context(tc.tile_pool(name="state", bufs=2))
            cpool = ctx.enter_context(tc.tile_pool(name="consts", bufs=1))
            ppool = ctx.enter_context(tc.tile_pool(name="psum", bufs=1, space="PSUM"))

            # PE warm-up: dummy matmuls on a memset tile into a trash PSUM
            # bank while the first DMAs are in flight — releases the HAM
            # clock throttle before the first real matmul arrives. memset on
            # gpsimd: that queue is live earliest, so the PE starts sooner.
            dummy = cpool.tile([128, 256], A_DT, tag="dummy")
            nc.gpsimd.memset(dummy[:], 0.0)
            trash = ppool.tile([128, 256], f32, tag="ps7")
            for i in range(WARMUP):
                nc.tensor.matmul(trash[:], dummy[:, 0:128], dummy[:],
                                 start=(i == 0), stop=(i == WARMUP - 1))

            # DMA triggers cost ~600ns of sequencer time each (DIRECT2D
            # descriptor generation), so the two HWDGE rings split the work:
            # sync carries the s-matrices, scalar carries W-columns + the
            # d-matrices. Everything else stays off those two queues early.
            t0 = cpool.tile([128, 2048], A_DT, tag="t0")
            nc.scalar.dma_start(t0[:, 0:1024], t0_dram[:])
            # layer-0 chunking: single-j-chunk first transfers so phase A
            # starts on the earliest possible bytes, quarters after
            L0_CHUNKS = [(0, 1024), (1024, 1024), (2048, 2048),
                         (4096, 2048), (6144, 2048)]
            s0 = apool.tile([128, JC * 1024], f8, tag="s8", bufs=KFP8, name="s0")
            for off, w in L0_CHUNKS:
                nc.sync.dma_start(s0[:, off:off + w], s8_dram[0, :, off:off + w])
            d0 = apool.tile([128, JC * 1024], f8, tag="d8", bufs=KFP8, name="d0")
            for off, w in L0_CHUNKS:
                nc.sync.dma_start(d0[:, off:off + w], d8_dram[0, :, off:off + w])
            scdc_all = cpool.tile([128, JC * SDW], A_DT, tag="scdc")
            nc.scalar.dma_start(scdc_all[:], scdc_dram[:])
            scdc = [scdc_all[:, jc * SDW:(jc + 1) * SDW] for jc in range(JC)]

            # bias accumulator: ACC[p, :] += sc[p]*state[p, :] + dc[p]*abs[p, :]
            # summed over all layers on DVE (axpys run during the matmul
            # phase). The output bias row is seeded into partition 0, so the
            # final fp16 ones-matmul partition-reduce yields bounds + bias in
            # one shot. Keeps all PE cycles for coefficient matmuls.
            acc = cpool.tile([128, 256], f32, tag="acc")
            nc.vector.memset(acc[:], 0.0)
            nc.scalar.dma_start(acc[0:1, :], bias_dram[:])
            ones16 = cpool.tile([128, 1], A_DT, tag="ones")
            nc.vector.memset(ones16[:], 1.0)
            acc16 = cpool.tile([128, 256], A_DT, tag="acc16")
            outsb = cpool.tile([1, 256], f32, tag="outsb")

            # |x0|T = max(x0, -x0) built on the DVE (idle early, and off the
            # trigger-carrying queues)
            for jc in range(JC):
                nc.vector.scalar_tensor_tensor(
                    t0[:, 1024 + jc * 128:1024 + (jc + 1) * 128],
                    t0[:, jc * 128:(jc + 1) * 128], -1.0,
                    t0[:, jc * 128:(jc + 1) * 128], MULT, MAX)

            # layer 0, phase A: P1[cc] = x0 @ S (128-wide moving operands)
            T1 = spool.tile([128, JC * 512], A_DT, tag="T", name="T1")
            psA = [ppool.tile([128, 128], f32, tag=f"ps{cc}", name=f"psA{cc}")
                   for cc in range(8)]
            for jc in range(JC):
                for cc in range(8):
                    nc.tensor.matmul(
                        psA[cc][:], s0[:, jc * 1024 + cc * 128:jc * 1024 + cc * 128 + 128],
                        t0[:, jc * 128:(jc + 1) * 128],
                        start=(jc == 0), stop=(jc == JC - 1))
            for cc in range(8):
                nc.scalar.activation(T1[:, cc * 512:cc * 512 + 128], psA[cc][:],
                                     mybir.ActivationFunctionType.Copy,
                                     scale=1.0 / F8_SCALE)
            # phase B: P2[cc] = |x0| @ Dm, then xl1 = P1+P2, xu1 = P1-P2
            psB = [ppool.tile([128, 128], f32, tag=f"ps{cc}", name=f"psB{cc}")
                   for cc in range(8)]
            for jc in range(JC):
                for cc in range(8):
                    nc.tensor.matmul(
                        psB[cc][:], d0[:, jc * 1024 + cc * 128:jc * 1024 + cc * 128 + 128],
                        t0[:, 1024 + jc * 128:1024 + (jc + 1) * 128],
                        start=(jc == 0), stop=(jc == JC - 1))
            for cc in range(8):
                nt = T1[:, cc * 512:(cc + 1) * 512]
                # nt[0:128] holds P1/s; xu = P1/s - P2/s, then xl = P1/s + P2/s
                nc.vector.scalar_tensor_tensor(
                    nt[:, 128:256], psB[cc][:], -1.0 / F8_SCALE, nt[:, 0:128],
                    MULT, ADD)
                nc.vector.scalar_tensor_tensor(
                    nt[:, 0:128], psB[cc][:], 1.0 / F8_SCALE, nt[:, 0:128],
                    MULT, ADD)
                nc.scalar.activation(nt[:, 256:384], nt[:, 0:128], ABS)
                nc.vector.scalar_tensor_tensor(
                    nt[:, 384:512], nt[:, 128:256], -1.0, nt[:, 128:256], MULT, MIN)
            T = T1

            # layer-0 bias round: xl = xu = x0, so 128-wide axpys with the
            # shared x0/|x0| inputs (col 2L+2 holds -dc0 for the upper half).
            # Emitted after the phase-B combines so the DVE queue doesn't
            # stall on the scdc arrival ahead of them.
            for jc in range(JC):
                x0s = t0[:, jc * 128:(jc + 1) * 128]
                abss = t0[:, 1024 + jc * 128:1024 + (jc + 1) * 128]
                nc.vector.scalar_tensor_tensor(
                    acc[:, 0:128], x0s, scdc[jc][:, 0:1], acc[:, 0:128], MULT, ADD)
                nc.vector.scalar_tensor_tensor(
                    acc[:, 128:256], x0s, scdc[jc][:, 0:1], acc[:, 128:256], MULT, ADD)
                nc.vector.scalar_tensor_tensor(
                    acc[:, 0:128], abss, scdc[jc][:, 1:2], acc[:, 0:128], MULT, ADD)
                nc.vector.scalar_tensor_tensor(
                    acc[:, 128:256], abss, scdc[jc][:, 2 * L + 2:2 * L + 3],
                    acc[:, 128:256], MULT, ADD)

            for l in range(1, L):
                fp8l = l < KFP8
                if fp8l:
                    s_t = apool.tile([128, JC * 1024], f8, tag="s8",
                                     bufs=KFP8, name=f"s{l}")
                    d_t = apool.tile([128, JC * 1024], f8, tag="d8",
                                     bufs=KFP8, name=f"d{l}")
                    s_src, d_src, li = s8_dram, d8_dram, l
                else:
                    s_t = apool.tile([128, JC * 1024], A_DT, tag="s", name=f"s{l}")
                    d_t = apool.tile([128, JC * 1024], A_DT, tag="d", name=f"d{l}")
                    s_src, d_src, li = s_dram, d_dram, l - KFP8
                # quarter-granularity (4 KB per partition row) measures the
                # best per-engine DMA bandwidth; both streams stay on the
                # sync ring — it carries nothing else, so triggers keep a
                # multi-layer prefetch lead
                for q in range(4):
                    nc.sync.dma_start(s_t[:, q * 2048:(q + 1) * 2048],
                                      s_src[li, :, q * 2048:(q + 1) * 2048])
                    nc.sync.dma_start(d_t[:, q * 2048:(q + 1) * 2048],
                                      d_src[li, :, q * 2048:(q + 1) * 2048])

                for jc in range(JC):
                    Ts = T[:, jc * 512:(jc + 1) * 512]
                    nc.vector.scalar_tensor_tensor(
                        acc[:], Ts[:, 0:256], scdc[jc][:, 2 * l:2 * l + 1],
                        acc[:], MULT, ADD)
                    nc.vector.scalar_tensor_tensor(
                        acc[:], Ts[:, 256:512], scdc[jc][:, 2 * l + 1:2 * l + 2],
                        acc[:], MULT, ADD)

                last = (l == L - 1)
                if last:
                    # acc is complete once this layer's bias round retires;
                    # the fp16 copy feeds the cheap ones-matmul reduce
                    nc.vector.tensor_copy(acc16[:], acc[:])

                newT = spool.tile([128, JC * 512], A_DT, tag="T", name=f"T{l + 1}")
                ps = [ppool.tile([128, 256], f32, tag=f"ps{i}", name=f"ps{i}_{l}")
                      for i in range(8)]
                # jc-outer for all but the last j-chunk: consumes DMA chunks
                # as they land, all 8 cc accumulation groups open in their
                # own PSUM banks
                for jc in range(JC - 1):
                    for cc in range(8):
                        off = jc * 1024 + cc * 128
                        nc.tensor.matmul(
                            ps[cc][:], s_t[:, off:off + 128],
                            T[:, jc * 512:jc * 512 + 256], start=(jc == 0), stop=False)
                        nc.tensor.matmul(
                            ps[cc][:], d_t[:, off:off + 128],
                            T[:, jc * 512 + 256:(jc + 1) * 512], start=False, stop=False)
                if last:
                    pb = ppool.tile([1, 256], f32, tag="ps0", name="pb")
                # last j-chunk cc-outer: staggers group closes so PSUM->SBUF
                # copies overlap the remaining matmuls; on the final layer
                # the input-bound matmuls interleave into the close stream
                jc = JC - 1
                for cc in range(8):
                    off = jc * 1024 + cc * 128
                    nc.tensor.matmul(
                        ps[cc][:], s_t[:, off:off + 128],
                        T[:, jc * 512:jc * 512 + 256], start=False, stop=False)
                    nc.tensor.matmul(
                        ps[cc][:], d_t[:, off:off + 128],
                        T[:, jc * 512 + 256:(jc + 1) * 512], start=False, stop=True)
                    nt = newT[:, cc * 512:(cc + 1) * 512]
                    if last:
                        nc.vector.tensor_copy(nt[:, 0:256], ps[cc][:])
                    elif fp8l:
                        nc.scalar.activation(nt[:, 0:256], ps[cc][:],
                                             mybir.ActivationFunctionType.Copy,
                                             scale=1.0 / F8_SCALE)
                    else:
                        nc.scalar.copy(nt[:, 0:256], ps[cc][:])
                    nc.scalar.activation(nt[:, 256:384], nt[:, 0:128], ABS)
                    nc.vector.scalar_tensor_tensor(
                        nt[:, 384:512], nt[:, 128:256], -1.0, nt[:, 128:256],
                        MULT, MIN)
                    if last:
                        if cc == 0:
                            nc.tensor.matmul(pb[:], ones16[:], acc16[:],
                                             start=True, stop=False)
                        else:
                            c = cc - 1
                            nc.tensor.matmul(
                                pb[:], scdc[c][:, 2 * L:2 * L + 1],
                                newT[:, c * 512:c * 512 + 256],
                                start=False, stop=False)
                            nc.tensor.matmul(
                                pb[:], scdc[c][:, 2 * L + 1:2 * L + 2],
                                newT[:, c * 512 + 256:(c + 1) * 512],
                                start=False, stop=False)
                T = newT

            c = JC - 1
            nc.tensor.matmul(pb[:], scdc[c][:, 2 * L:2 * L + 1],
                             T[:, c * 512:c * 512 + 256], start=False, stop=False)
            nc.tensor.matmul(pb[:], scdc[c][:, 2 * L + 1:2 * L + 2],
                             T[:, c * 512 + 256:(c + 1) * 512], start=False, stop=True)
            nc.vector.tensor_copy(outsb[:], pb[:])
            nc.sync.dma_start(out_dram[:], outsb[:])
    nc.compile()
    return nc


def _prep_inputs(weights, biases, net_x_lowers, net_x_uppers,
                 net_b_lowers, net_b_uppers, input_lowers, input_uppers):
    W = np.ascontiguousarray(np.asarray(weights, dtype=np.float32))
    b = np.asarray(biases, dtype=np.float32).reshape(D)
    AL = np.asarray(net_x_lowers, dtype=np.float32)
    AU = np.asarray(net_x_uppers, dtype=np.float32)
    cL = np.asarray(net_b_lowers, dtype=np.float32).reshape(L, D)
    cU = np.asarray(net_b_uppers, dtype=np.float32).reshape(L, D)
    lo = np.asarray(input_lowers, dtype=np.float32).reshape(D)
    up = np.asarray(input_uppers, dtype=np.float32).reshape(D)

    import ml_dtypes
    S = 0.5 * (AL + AU)
    Dm = 0.5 * (AL - AU)
    # (L, 128, JC*1024): [l, p, jc*1024 + c] = S[l, jc*128 + p, c]
    s_all = np.ascontiguousarray(
        S.reshape(L, JC, 128, D).transpose(0, 2, 1, 3).reshape(L, 128, JC * D))
    d_all = np.ascontiguousarray(
        Dm.reshape(L, JC, 128, D).transpose(0, 2, 1, 3).reshape(L, 128, JC * D))
    s8_mats = (s_all[:KFP8] * F8_SCALE).astype(
        ml_dtypes.float8_e3m4).view(np.uint8)
    d8_mats = (d_all[:KFP8] * F8_SCALE).astype(
        ml_dtypes.float8_e3m4).view(np.uint8)
    s_mats = s_all[KFP8:].astype(A_NP)
    d_mats = d_all[KFP8:].astype(A_NP)

    sc = 0.5 * (cL + cU)
    dc = 0.5 * (cL - cU)
    s_in = 0.5 * (lo + up)
    d_in = 0.5 * (lo - up)
    sd = np.empty((JC, 128, SDW), np.float32)
    sd[:, :, 0:2 * L:2] = sc.reshape(L, JC, 128).transpose(1, 2, 0)
    sd[:, :, 1:2 * L:2] = dc.reshape(L, JC, 128).transpose(1, 2, 0)
    sd[:, :, 2 * L] = s_in.reshape(JC, 128)
    sd[:, :, 2 * L + 1] = d_in.reshape(JC, 128)
    sd[:, :, 2 * L + 2] = -dc[0].reshape(JC, 128)
    # dram layout (128, JC*SDW): [p, jc*SDW + col]
    scdc = np.ascontiguousarray(
        sd.transpose(1, 0, 2).reshape(128, JC * SDW)).astype(A_NP)

    Wh = W.astype(A_NP)  # x0T[j, r] = W[j, r]; round once
    Wr = Wh.reshape(JC, 128, D)
    in_maps = []
    for k in range(NCORES):
        # (128, 1024): [p, jc*128 + r] = W[jc*128 + p, k*R + r]
        cols = np.ascontiguousarray(
            Wr[:, :, k * R:(k + 1) * R].transpose(1, 0, 2).reshape(128, JC * R))
        b0 = np.empty((1, 256), np.float32)
        b0[0, 0:128] = b[k * R:(k + 1) * R]
        b0[0, 128:256] = b[k * R:(k + 1) * R]
        in_maps.append({
            "s8_mats": s8_mats,
            "d8_mats": d8_mats,
            "s_mats": s_mats,
            "d_mats": d_mats,
            "scdc": scdc,
            "t0cols": cols,
            "bias0": b0,
        })
    return in_maps


def _run(inputs, trace=False):
    if "nc" not in _CACHE:
        _CACHE["nc"] = _build()
    nc = _CACHE["nc"]
    in_maps = _prep_inputs(**inputs)
    try:
        res = run_bass_kernel_spmd(nc, in_maps, core_ids=list(range(NCORES)),
                                   trace=trace)
    except Exception:
        # transient NRT device errors have been observed; retry once
        res = run_bass_kernel_spmd(nc, in_maps, core_ids=list(range(NCORES)),
                                   trace=trace)
    lowers = np.empty((D, 1), np.float32)
    uppers = np.empty((D, 1), np.float32)
    for k in range(NCORES):
        arr = res.results[k]["out"]
        lowers[k * R:(k + 1) * R, 0] = arr[0, 0:128]
        uppers[k * R:(k + 1) * R, 0] = arr[0, 128:256]
    out = np.stack([lowers, uppers])
    return out, res


def kernel(**inputs):
    out, _ = _run(inputs, trace=False)
    return out
